# revision 10
# baseline (speedup 1.0000x reference)
"""1-NN min-Euclidean-distance kernel for Trainium2 (8 NeuronCores, SPMD).

Problem: queries [8192, 96] f32, train [65536, 96] f32 ->
         out[q] = min_t ||q - t||_2 * 10  (f32 [8192])

Strategy (per core, queries sharded 1024/core, train replicated):
  z[q,t] = ||t||^2 - 2*q.t  is computed as a single K=98 matmul:
    lhsT rows 0..95 = -2*q_d (fp16), rows 96,97 = 1.0
    rhs  rows 0..95 = t_d   (fp16), rows 96,97 = y2_hi, y2_lo (hi/lo split
    of ||t||^2 so the fp16 rhs carries ~fp32 precision for the norm term)
  min_t z is accumulated with chained tensor_tensor_reduce ops: each TTR
  consumes one PSUM tile (fp32) and one ACT-pre-copied SBUF tile (fp16)
  with op0=min, op1=min, carrying the per-partition running min in a
  [128,1] accumulator column.  Finally out = sqrt(max(x2 + min_z, 0)) * 10
  with one Newton step to clean up the ACT sqrt table error.

The train matrix must be presented with dims on partitions ([98, 65536]
fp16).  Each core PE-transposes 1/8 of the train set ([8192, 96]) in a
pre-pass and the slices are exchanged with an AllGather (distributed
mode), or each core transposes everything locally (fallback mode).
"""

import numpy as np

import concourse.bass as bass
import concourse.mybir as mybir
import concourse.tile as tile
from concourse.masks import make_identity
from concourse.vector_clock import ScopedClock


class AwsTileContext(tile.TileContext):
    """TileContext whose kernel-tail drain is AWS-walrus-compatible.

    Stock Tile attaches one sem-wait per ticked logical processor to the
    single kernel-tail Drain; the neuronxcc walrus_driver in this container
    (CoreV3GenImpl setupSyncWait) only accepts one sync wait on a CTRL
    instruction.  Emit the waits on a chain of sync-engine NOPs (in-order
    queue, one wait each) and leave the Drain waitless instead.
    """

    def _drain_and_barrier(self, tick_clock, wait_clock):
        nc = self.nc
        carrier = nc.sync.nop()
        wait_clock.add_sem_waits(
            carrier.ins, ScopedClock({None: tick_clock.global_clock})
        )
        waits = list(carrier.ins.sync_info.on_wait)
        carrier.ins.sync_info.on_wait = waits[:1]
        for wobj in waits[1:]:
            n = nc.sync.nop()
            if n.ins.sync_info is None:
                n.ins.sync_info = mybir.SyncInfo(on_wait=[wobj], on_update=[])
            else:
                n.ins.sync_info.on_wait = [wobj]
        nc.sync.drain()
        nc.all_engine_barrier()
        assert self.sems is not None
        popped = nc._tile_sem_poison_stack.pop()
        assert popped is self._sem_poison
        nc.clear_and_free_semaphores(list(self.sems.allocated().values()))
        nc.all_engine_barrier()

F32 = mybir.dt.float32
F16 = mybir.dt.float16
ALU = mybir.AluOpType
AFT = mybir.ActivationFunctionType

# The container's neuronxcc walrus (CoreV2/V3GenImpl::setupSyncWait) caps
# sync waits per instruction; the cap varies by ISA struct and is 1 for
# most types we emit (DMA pseudo-ops, Drain, TensorCopy, ...).  NOP was
# verified to accept at least 9.  Cap everything at 1 except NOP.
_MULTIWAIT_OK = {"NoOp"}


def _split_excess_waits(nc: bass.Bass) -> int:
    """Rewrite instructions carrying more sem waits than the AWS walrus
    allows: move the excess onto same-engine NOPs inserted just before
    (engine queues are in-order, so the waits still settle first)."""
    n_split = 0
    for fn in nc.m.functions:
        for blk in fn.blocks:
            insts = list(blk.instructions)
            out = []
            changed = False
            for inst in insts:
                si = inst.sync_info
                cap = 8 if inst.opcode in _MULTIWAIT_OK else 1
                if si is not None and len(si.on_wait) > cap:
                    waits = list(si.on_wait)
                    movable = [w for w in waits if w.wait_reg is None]
                    pinned = [w for w in waits if w.wait_reg is not None]
                    keep_n = max(cap - len(pinned), 0)
                    keep, excess = movable[:keep_n], movable[keep_n:]
                    if excess:
                        for w in excess:
                            nop = mybir.InstNoOp(
                                name=f"I-waitsplit-{nc.next_id()}",
                                opcode="NoOp",
                                engine=inst.engine,
                                ins=[],
                                outs=[],
                            )
                            nop.sync_info = mybir.SyncInfo(
                                on_wait=[w], on_update=[]
                            )
                            nc.register_instruction(nop)
                            out.append(nop)
                            n_split += 1
                        si.on_wait = pinned + keep
                        changed = True
                out.append(inst)
            if changed:
                blk.instructions = out
    return n_split

N_CORES = 8
P = 128


def build_nc(
    nq_c: int = 1024,  # queries per core
    nt: int = 65536,  # total train points
    d: int = 96,  # feature dim
    distributed: bool = True,
    unit: int = 1024,  # drain unit (columns per PSUM tile, 2 banks f32)
    tc_pre: int = 16,  # train tiles of 128 per pre-pass staging chunk
    n_cores: int = N_CORES,
):
    k = d + 2
    qt = nq_c // P  # query tiles per core
    assert nq_c % P == 0 and nt % (P * tc_pre) == 0
    nt_c = nt // n_cores if distributed else nt  # trains transposed per core
    assert nt_c % (P * tc_pre) == 0
    blk = nt // n_cores  # column block size of the resident rhs [k, nb, blk]
    nb = n_cores
    assert blk % unit == 0 and unit % 512 == 0

    nc = bass.Bass(num_devices=n_cores, enable_partition_id=True)

    q_ext = nc.dram_tensor("q", [nq_c, d], F32, kind="ExternalInput")
    t_ext = nc.dram_tensor(
        "train", [nt_c, d], F32, kind="ExternalInput"
    )  # per-core slice (distributed) or full set
    out_ext = nc.dram_tensor("out", [nq_c], F32, kind="ExternalOutput")

    with AwsTileContext(nc) as tc:
        with tc.tile_pool(name="singles", bufs=1) as singles:
            identity = singles.tile([P, P], F16)
            make_identity(nc, identity)
            # resident transposed+augmented train matrix, viewed as nb blocks
            t_aug = singles.tile([k, nb, blk], F16)
            lhsT_all = singles.tile([k, qt, P], F16)
            x2s = singles.tile([P, qt], F32)
            finals = singles.tile([P, qt], F32)

            # ---------------- phase 0: query prep ----------------
            with (
                tc.tile_pool(name="qprep", bufs=1) as qp,
                tc.tile_pool(name="qpsum", bufs=2, space="PSUM") as qpsum,
            ):
                q32 = qp.tile([P, qt, d], F32)
                nc.sync.dma_start(
                    out=q32, in_=q_ext.rearrange("(m p) d -> p m d", p=P)
                )
                q16 = qp.tile([P, qt, d], F16)
                nc.vector.tensor_copy(q16, q32)
                sqq = qp.tile([P, qt, d], F32)
                nc.vector.tensor_mul(sqq, q16, q16)
                nc.vector.tensor_reduce(
                    x2s, sqq, axis=mybir.AxisListType.X, op=ALU.add
                )
                aug_q = qp.tile([P, qt, k], F16)
                nc.vector.memset(aug_q, 1.0)
                nc.vector.tensor_scalar_mul(aug_q[:, :, 0:d], q16, -2.0)
                for m in range(qt):
                    pt = qpsum.tile([k, P], F16, tag="pt")
                    nc.tensor.transpose(pt, aug_q[:, m : m + 1, :], identity)
                    nc.vector.tensor_copy(lhsT_all[:, m : m + 1, :], pt)

            # ---------------- phase 1: train transpose ----------------
            with (
                tc.tile_pool(name="tprep", bufs=2) as tp,
                tc.tile_pool(name="tpsum", bufs=4, space="PSUM") as tpsum,
                tc.tile_pool(name="tdram", bufs=1, space="DRAM") as tdram,
            ):
                if distributed:
                    t_loc = tp.tile([k, nt_c], F16)  # local transposed slice
                n_chunks = nt_c // (P * tc_pre)
                t_r = t_ext.rearrange("(c i p) d -> c p i d", p=P, i=tc_pre)
                for c in range(n_chunks):
                    tr32 = tp.tile([P, tc_pre, d], F32)
                    nc.sync.dma_start(out=tr32, in_=t_r[c : c + 1])
                    tr16 = tp.tile([P, tc_pre, d], F16)
                    nc.vector.tensor_copy(tr16, tr32)
                    sq32 = tp.tile([P, tc_pre, d], F32)
                    nc.scalar.activation(sq32, tr16, AFT.Square)
                    y2 = tp.tile([P, tc_pre], F32)
                    nc.vector.tensor_reduce(
                        y2, sq32, axis=mybir.AxisListType.X, op=ALU.add
                    )
                    y2h = tp.tile([P, tc_pre], F16)
                    nc.vector.tensor_copy(y2h, y2)
                    y2h32 = tp.tile([P, tc_pre], F32)
                    nc.vector.tensor_copy(y2h32, y2h)
                    y2l = tp.tile([P, tc_pre], F32)
                    nc.vector.tensor_sub(y2l, y2, y2h32)
                    aug_t = tp.tile([P, tc_pre, k], F16)
                    nc.vector.tensor_copy(aug_t[:, :, 0:d], tr16)
                    nc.vector.tensor_copy(aug_t[:, :, d : d + 1], y2h)
                    nc.vector.tensor_copy(aug_t[:, :, d + 1 : d + 2], y2l)
                    for i in range(tc_pre):
                        col = (c * tc_pre + i) * P
                        pt2 = tpsum.tile([k, P], F16, tag="pt2")
                        nc.tensor.transpose(
                            pt2, aug_t[:, i : i + 1, :], identity
                        )
                        if distributed:
                            dst = t_loc[:, col : col + P]
                        else:
                            b, o = divmod(col, blk)
                            dst = t_aug[:, b : b + 1, o : o + P]
                        # alternate copy engine to balance DVE/ACT load
                        if i % 3 == 2:
                            nc.scalar.activation(dst, pt2, AFT.Copy)
                        else:
                            nc.vector.tensor_copy(dst, pt2)

                if distributed:
                    t_loc_dram = tdram.tile([k, nt_c], F16)
                    nc.sync.dma_start(out=t_loc_dram, in_=t_loc)
                    t_full_dram = tdram.tile(
                        [nb, k, blk], F16, addr_space="Shared"
                    )
                    nc.gpsimd.collective_compute(
                        "AllGather",
                        ALU.bypass,
                        replica_groups=[list(range(n_cores))],
                        ins=[t_loc_dram[:]],
                        outs=[t_full_dram[:]],
                    )
                    for b in range(nb):
                        nc.sync.dma_start(
                            out=t_aug[:, b : b + 1, :],
                            in_=t_full_dram[b : b + 1],
                        )

            # ---------------- phase 2: distance matmuls + min drain ----------------
            n_units = nt // unit  # per q-tile
            assert n_units % 2 == 0
            n_pairs = n_units // 2
            mm_per_unit = unit // 512
            with (
                tc.tile_pool(name="zdrain", bufs=3) as zd,
                tc.tile_pool(name="mpsum", bufs=3, space="PSUM") as mpsum,
            ):
                for m in range(qt):
                    prev = None
                    for u in range(n_units):
                        col = u * unit
                        b, o = divmod(col, blk)
                        pz = mpsum.tile([P, unit], F32, tag="pz")
                        for j in range(mm_per_unit):
                            nc.tensor.matmul(
                                pz[:, j * 512 : (j + 1) * 512],
                                lhsT_all[:, m : m + 1, :],
                                t_aug[:, b : b + 1, o + j * 512 : o + (j + 1) * 512],
                                start=True,
                                stop=True,
                            )
                        if u % 2 == 0:
                            zc = zd.tile([P, unit], F16, tag="zc")
                            nc.scalar.activation(zc, pz, AFT.Copy)
                            pending = zc
                        else:
                            # running min of both tiles via free-dim scan;
                            # the last column carries the chained minimum
                            scan = zd.tile([P, unit], F32, tag="scan")
                            init = (
                                3.0e38
                                if prev is None
                                else prev[:, unit - 1 : unit]
                            )
                            nc.vector.tensor_tensor_scan(
                                out=scan,
                                data0=pz,
                                data1=pending,
                                initial=init,
                                op0=ALU.min,
                                op1=ALU.min,
                            )
                            prev = scan
                    nc.vector.tensor_copy(
                        finals[:, m : m + 1], prev[:, unit - 1 : unit]
                    )

            # ---------------- phase 3: epilogue ----------------
            with tc.tile_pool(name="ep", bufs=1) as ep:
                sq = ep.tile([P, qt], F32)
                nc.vector.tensor_add(sq, finals, x2s)
                sqc = ep.tile([P, qt], F32)
                nc.vector.tensor_scalar_max(sqc, sq, 1.0e-30)
                s0 = ep.tile([P, qt], F32)
                nc.scalar.activation(s0, sqc, AFT.Sqrt)
                inv = ep.tile([P, qt], F32)
                nc.vector.reciprocal(inv, s0)
                t1 = ep.tile([P, qt], F32)
                nc.vector.tensor_mul(t1, sqc, inv)
                s1 = ep.tile([P, qt], F32)
                nc.vector.tensor_add(s1, s0, t1)
                d10 = ep.tile([P, qt], F32)
                nc.vector.tensor_scalar_mul(d10, s1, 5.0)
                nc.sync.dma_start(
                    out=out_ext.rearrange("(m p) -> p m", p=P), in_=d10
                )

    _split_excess_waits(nc)
    return nc


_NC_CACHE: dict = {}


def _get_nc(key):
    if key not in _NC_CACHE:
        nq_c, nt, d, distributed = key
        _NC_CACHE[key] = build_nc(
            nq_c=nq_c, nt=nt, d=d, distributed=distributed
        )
    return _NC_CACHE[key]


import os as _os

DISTRIBUTED = _os.environ.get("KNN_DIST", "1") == "1"


def kernel(mutation_dist: np.ndarray, train_data: np.ndarray) -> np.ndarray:
    from concourse.bass_utils import run_bass_kernel_spmd

    q = np.ascontiguousarray(np.asarray(mutation_dist, dtype=np.float32))
    t = np.ascontiguousarray(np.asarray(train_data, dtype=np.float32))
    nq, d = q.shape
    nt, d2 = t.shape
    assert d == d2
    nq_c = nq // N_CORES
    nt_c = nt // N_CORES

    nc = _get_nc((nq_c, nt, d, DISTRIBUTED))
    in_maps = []
    for c in range(N_CORES):
        m = {"q": q[c * nq_c : (c + 1) * nq_c]}
        if DISTRIBUTED:
            m["train"] = np.ascontiguousarray(t[c * nt_c : (c + 1) * nt_c])
        else:
            m["train"] = t
        in_maps.append(m)

    res = run_bass_kernel_spmd(nc, in_maps, list(range(N_CORES))).results
    return np.concatenate(
        [np.asarray(res[c]["out"], dtype=np.float32) for c in range(N_CORES)]
    )
